# revision 3
# baseline (speedup 1.0000x reference)
"""Single-head causal attention (B=8, T=2048, C=1024, H=64) on 8 TRN2 NeuronCores.

Strategy: pure data parallelism — batch element b runs on core b. Each core
computes, for its [T, C] slices q_b / k_b:

    Q = q_b @ Wq ; K = k_b @ Wk ; V = k_b @ Wv          (projections)
    S = Q @ K^T / sqrt(C), causal-masked ; P = exp(S)    (no max-subtract:
    out = (P @ V) / (P @ 1)                               S is well-scaled)

Device-side layout (all matmuls bf16, fp32 PSUM accumulation):
  * Host pre-transposes q/k to [C, T] and pre-blocks them [tb, p, c, t] so
    each 512-column block arrives as contiguous DMA lines with the
    contraction dim (C) on SBUF partitions — zero on-chip input transposes.
  * Weights are DMA'd FIRST on the sync ring so the first projection can
    start as soon as the first k block lands.
  * Q projection uses a duplicated stationary [Wq | Wq], so Q^T comes out
    replicated on partition halves 0:64 / 64:128 in one pass.  K^T is
    likewise replicated to partitions 64:128 by a small SBUF->SBUF DMA.
  * Scores run as ROW-TILED PAIRS: chunk for key-tile j uses PE rows 0:63
    (operands on partitions 0:64), chunk j+1 uses rows 64:127 (operands on
    partitions 64:128).  The two matmuls execute concurrently in the array
    (contraction is only H=64), doubling score throughput.
  * Each pair's two PSUM banks are exp'd by ONE wide scalar-engine
    activation (1/sqrt(C) folded into the activation scale), halving the
    per-instruction ACT overhead.
  * Softmax denominators come free via a ones column appended to V (row 0
    of the PV accumulator is P @ 1); normalization multiplies rows 1:65 by
    the broadcast reciprocal and stores bf16 output (host casts to f32).
"""

import numpy as np
import ml_dtypes

B, T, C, H = 8, 2048, 1024, 64
P = 128                  # SBUF partitions
CCH = C // P             # 8 contraction chunks
NJ = T // P              # 16 key tiles of 128
NB = T // 512            # 4 column blocks of 512
SCALE = float(C) ** -0.5

_cached = {}


def _build():
    import concourse.bass as bass
    import concourse.mybir as mybir
    import concourse.tile as tile
    from concourse import bacc

    dt = mybir.dt
    nc = bacc.Bacc("TRN2", target_bir_lowering=False, debug=False, num_devices=B)

    # blocked inputs: [tb, p, c, t] so each 512-col block is contiguous
    qT = nc.dram_tensor("qT", [NB, P, CCH, 512], dt.bfloat16, kind="ExternalInput").ap()
    kT = nc.dram_tensor("kT", [NB, P, CCH, 512], dt.bfloat16, kind="ExternalInput").ap()
    wq = nc.dram_tensor("wq", [P, CCH, P], dt.bfloat16, kind="ExternalInput").ap()
    wkv = nc.dram_tensor("wkv", [P, CCH, P], dt.bfloat16, kind="ExternalInput").ap()
    dmask = nc.dram_tensor("dmask", [P, P], dt.bfloat16, kind="ExternalInput").ap()
    idb = nc.dram_tensor("idb", [P, P], dt.bfloat16, kind="ExternalInput").ap()
    out_t = nc.dram_tensor("out_t", [H, T], dt.bfloat16, kind="ExternalOutput").ap()

    EXP = mybir.ActivationFunctionType.Exp

    with tile.TileContext(nc) as tc:
        with (
            tc.tile_pool(name="consts", bufs=1) as consts,
            tc.tile_pool(name="inbuf", bufs=1) as inbuf,
            tc.tile_pool(name="proj", bufs=1) as proj,
            tc.tile_pool(name="projpsum", bufs=1, space="PSUM") as projpsum,
            tc.tile_pool(name="spsum", bufs=2, space="PSUM") as spsum,
            tc.tile_pool(name="opsum", bufs=1, space="PSUM") as opsum,
            tc.tile_pool(name="pbuf", bufs=3) as pbuf,
            tc.tile_pool(name="ebuf", bufs=2) as ebuf,
        ):
            # ---- constants: weights FIRST on the sync ring ------------------
            mask_s = consts.tile([P, P], dt.bfloat16)
            idb_s = consts.tile([P, P], dt.bfloat16)
            wq_s = consts.tile([P, CCH, P], dt.bfloat16)
            wkv_s = consts.tile([P, CCH, P], dt.bfloat16)
            nc.sync.dma_start(out=wkv_s[:], in_=wkv[:])
            nc.sync.dma_start(out=wq_s[:], in_=wq[:])
            nc.sync.dma_start(out=mask_s[:], in_=dmask[:])
            nc.sync.dma_start(out=idb_s[:], in_=idb[:])

            kT_s = inbuf.tile([P, NB, CCH, 512], dt.bfloat16)
            qT_s = inbuf.tile([P, NB, CCH, 512], dt.bfloat16)
            KVT_s = proj.tile([P, T], dt.bfloat16)   # rows 0:64 K^T, 64:128 V^T
            QT_s = proj.tile([P, T], dt.bfloat16)    # Q^T duplicated both halves
            KTD_s = proj.tile([P, T], dt.bfloat16)   # rows 64:128 = K^T dup
            V1_s = proj.tile([P, NJ, 66], dt.bfloat16)  # ones col + V natural
            nc.vector.memset(V1_s[:, :, 0:1], 1.0)

            # ---- input DMAs, all upfront, half-block granularity ------------
            for tb in range(NB):
                nc.sync.dma_start(out=kT_s[:, tb, 0:4], in_=kT[tb, :, 0:4])
                nc.sync.dma_start(out=kT_s[:, tb, 4:8], in_=kT[tb, :, 4:8])
                nc.sync.dma_start(out=qT_s[:, tb, 0:4], in_=qT[tb, :, 0:4])
                nc.sync.dma_start(out=qT_s[:, tb, 4:8], in_=qT[tb, :, 4:8])

            # ---- pipeline stages --------------------------------------------
            def proj_block(tb):
                """Project one 512-col block of k (K^T/V^T) and q (Q^T dup)."""
                sl = slice(512 * tb, 512 * (tb + 1))
                KVp = projpsum.tile([P, 512], dt.float32, tag="kv")
                for c in range(CCH):
                    nc.tensor.matmul(KVp[:], lhsT=wkv_s[:, c, :],
                                     rhs=kT_s[:, tb, c, :],
                                     start=(c == 0), stop=(c == CCH - 1))
                nc.vector.tensor_copy(out=KVT_s[:, sl], in_=KVp[:])
                # replicate K^T onto partitions 64:128 for row-tiled scores
                nc.scalar.dma_start(out=KTD_s[64:128, sl], in_=KVT_s[0:64, sl])

                Qp = projpsum.tile([P, 512], dt.float32, tag="q")
                for c in range(CCH):
                    nc.tensor.matmul(Qp[:], lhsT=wq_s[:, c, :],
                                     rhs=qT_s[:, tb, c, :],
                                     start=(c == 0), stop=(c == CCH - 1))
                nc.vector.tensor_copy(out=QT_s[:, sl], in_=Qp[:])

                for jj in range(4):
                    j = 4 * tb + jj
                    vtp = projpsum.tile([P, P], dt.bfloat16, tag="vt")
                    nc.tensor.transpose(
                        vtp[:], KVT_s[:, P * j:P * (j + 1)], idb_s[:])
                    nc.vector.tensor_copy(out=V1_s[:, j, 1:65], in_=vtp[:, 64:128])

            def attn_block(ic):
                """Score/exp/accumulate + normalize/store one 512-col i-block.

                Key tiles j are processed in row-tiled pairs (2p, 2p+1): the
                even chunk computes on PE rows 0:63 (operands at partitions
                0:64), the odd chunk on rows 64:127 (operands at 64:128)."""
                ilo = 512 * ic
                ihi = 512 * (ic + 1)
                nj = 4 * ic + 4
                OUTp = opsum.tile([H + 1, 512], dt.float32, tag="out")
                for p_ in range(nj // 2):
                    jA, jB = 2 * p_, 2 * p_ + 1
                    loA = max(P * jA, ilo)
                    loB = max(P * jB, ilo)
                    wA = ihi - loA
                    wB = ihi - loB
                    Sp = spsum.tile([P, 1024], dt.float32, tag="s")
                    nc.tensor.matmul(Sp[:, 0:wA],
                                     lhsT=KVT_s[0:H, P * jA:P * (jA + 1)],
                                     rhs=QT_s[0:H, loA:loA + wA],
                                     start=True, stop=True)
                    nc.tensor.matmul(Sp[:, 512:512 + wB],
                                     lhsT=KTD_s[64:128, P * jB:P * (jB + 1)],
                                     rhs=QT_s[64:128, loB:loB + wB],
                                     start=True, stop=True)
                    Pt = pbuf.tile([P, 1024], dt.bfloat16, tag="p")
                    nc.scalar.activation(out=Pt[:, 0:512 + wB],
                                         in_=Sp[:, 0:512 + wB],
                                         func=EXP, scale=SCALE)
                    if jA >= 4 * ic:
                        # diagonal tile: zero strictly-lower 128x128 triangle
                        nc.vector.tensor_mul(Pt[:, 0:P], Pt[:, 0:P], mask_s[:])
                    if jB >= 4 * ic:
                        nc.vector.tensor_mul(Pt[:, 512:512 + P],
                                             Pt[:, 512:512 + P], mask_s[:])
                    nc.tensor.matmul(OUTp[:, loA - ilo:512],
                                     lhsT=V1_s[:, jA, 0:65],
                                     rhs=Pt[:, 0:wA],
                                     start=(p_ == 0), stop=False)
                    nc.tensor.matmul(OUTp[:, loB - ilo:512],
                                     lhsT=V1_s[:, jB, 0:65],
                                     rhs=Pt[:, 512:512 + wB],
                                     start=False, stop=(p_ == nj // 2 - 1))

                # normalize rows 1:65 by row 0 (l) and store bf16 out^T block
                linv = ebuf.tile([1, 512], dt.float32, tag="l")
                nc.vector.reciprocal_approx_fast(linv[:], OUTp[0:1, :])
                lbc = ebuf.tile([H + 1, 512], dt.float32, tag="b")
                nc.gpsimd.partition_broadcast(lbc[:], linv[:])
                ot = ebuf.tile([H + 1, 512], dt.bfloat16, tag="o")
                nc.vector.tensor_mul(ot[:], OUTp[:], lbc[:])
                nc.sync.dma_start(out=out_t[:, ilo:ilo + 512], in_=ot[1:H + 1, :])

            for blk in range(NB):
                proj_block(blk)
                attn_block(blk)

    nc.compile()
    return nc


def _get_nc():
    if "nc" not in _cached:
        _cached["nc"] = _build()
    return _cached["nc"]


def _block(xT):
    """[C, T] -> [NB, P, CCH, 512] so each 512-col block is contiguous."""
    return np.ascontiguousarray(
        xT.reshape(CCH, P, NB, 512).transpose(2, 1, 0, 3))


def _wblock(w):
    """[C, Hw] -> [P, CCH, Hw] contiguous (contraction chunks on partitions)."""
    return np.ascontiguousarray(
        w.reshape(CCH, P, w.shape[1]).transpose(1, 0, 2))


def _host_inputs(q, k, Wq, Wk, Wv):
    bf16 = ml_dtypes.bfloat16
    wq_h = _wblock(np.concatenate([Wq, Wq], axis=1).astype(bf16))
    wkv_h = _wblock(np.concatenate([Wk, Wv], axis=1).astype(bf16))
    dmask_h = np.triu(np.ones((P, P), dtype=np.float32)).astype(bf16)
    idb_h = np.eye(P, dtype=np.float32).astype(bf16)
    in_maps = []
    for b in range(B):
        in_maps.append({
            "qT": _block(q[b].T.astype(bf16)),
            "kT": _block(k[b].T.astype(bf16)),
            "wq": wq_h,
            "wkv": wkv_h,
            "dmask": dmask_h,
            "idb": idb_h,
        })
    return in_maps


def kernel(q, k, Wq, Wk, Wv):
    from concourse.bass_utils import run_bass_kernel_spmd

    nc = _get_nc()
    in_maps = _host_inputs(q, k, Wq, Wk, Wv)
    res = run_bass_kernel_spmd(nc, in_maps, list(range(B)))
    return np.stack(
        [res.results[b]["out_t"].T for b in range(B)]).astype(np.float32)


if __name__ == "__main__":
    rng = np.random.default_rng(0)
    q = rng.standard_normal((B, T, C)).astype(np.float32)
    k = rng.standard_normal((B, T, C)).astype(np.float32)
    Wq = (rng.standard_normal((C, H)) * 0.02).astype(np.float32)
    Wk = (rng.standard_normal((C, H)) * 0.02).astype(np.float32)
    Wv = (rng.standard_normal((C, H)) * 0.02).astype(np.float32)
    o = kernel(q, k, Wq, Wk, Wv)
    print("out", o.shape, o.dtype, float(np.abs(o).max()))


# revision 5
# speedup vs baseline: 1.0679x; 1.0679x over previous
"""Single-head causal attention (B=8, T=2048, C=1024, H=64) on 8 TRN2 NeuronCores.

Strategy: pure data parallelism — batch element b runs on core b. Each core
computes, for its [T, C] slices q_b / k_b:

    Q = q_b @ Wq ; K = k_b @ Wk ; V = k_b @ Wv          (projections)
    S = Q @ K^T / sqrt(C), causal-masked ; P = exp(S)    (no max-subtract:
    out = (P @ V) / (P @ 1)                               S is well-scaled)

Device-side layout:
  * Host pre-transposes q/k to [C, T] and pre-blocks them [tb, p, c, t];
    contraction (C) lands on SBUF partitions, zero on-chip input transposes.
    k ships bf16 (feeds K and V);  q ships fp8-e4m3 and Wq ships fp8
    pre-scaled by 64 (1/64 folded into the exp scale), so the Q projection
    runs in DoubleRow mode (2 contraction chunks per matmul, 2x rate).
  * Weights+identity are DMA'd FIRST; ~3us of identity warm-up matmuls run
    while the first k block streams in, so the PE HAM clock-gate reaches
    8/8 before real work starts.  DMA order retires the k3-dependent tail
    last (q3 early), and attention for the last i-block is spread through
    the kernel into an SBUF accumulator so the endgame exp burst is gone.
  * Q projection uses a duplicated stationary [Wq | Wq], so Q^T comes out
    replicated on partition halves 0:64 / 64:128 in one pass.  K^T is
    likewise replicated to partitions 64:128 by a small SBUF->SBUF DMA.
  * Scores run as ROW-TILED PAIRS: chunk for key-tile j uses PE rows 0:63
    (operands on partitions 0:64), chunk j+1 rows 64:127 (operands on
    partitions 64:128); the two matmuls execute concurrently (contraction
    is only H=64), doubling score throughput.  One wide scalar-engine
    activation exps both PSUM banks of a pair.
  * Softmax denominators come free via a ones column appended to V (row 0
    of the PV accumulator is P @ 1).  Outputs leave PSUM by direct
    PSUM->DRAM DMA, unnormalized [l; out^T] fp32; the host divides.
"""

import numpy as np
import ml_dtypes

B, T, C, H = 8, 2048, 1024, 64
P = 128                  # SBUF partitions
CCH = C // P             # 8 contraction chunks
NJ = T // P              # 16 key tiles of 128
NB = T // 512            # 4 column blocks of 512
SCALE = float(C) ** -0.5
QS = 64.0                # fp8 Wq pre-scale (folded out via exp scale)
NWARM = 16               # HAM warm-up matmuls

_cached = {}


def _build():
    import concourse.bass as bass
    import concourse.mybir as mybir
    import concourse.tile as tile
    from concourse import bacc

    dt = mybir.dt
    nc = bacc.Bacc("TRN2", target_bir_lowering=False, debug=False, num_devices=B)

    qT = nc.dram_tensor("qT", [NB, P, CCH, 512], dt.float8e4, kind="ExternalInput").ap()
    kT = nc.dram_tensor("kT", [NB, P, CCH, 512], dt.bfloat16, kind="ExternalInput").ap()
    wq = nc.dram_tensor("wq", [P, CCH, P], dt.float8e4, kind="ExternalInput").ap()
    wkv = nc.dram_tensor("wkv", [P, CCH, P], dt.bfloat16, kind="ExternalInput").ap()
    dmask = nc.dram_tensor("dmask", [P, P], dt.bfloat16, kind="ExternalInput").ap()
    idb = nc.dram_tensor("idb", [P, P], dt.bfloat16, kind="ExternalInput").ap()
    # unnormalized [l ; out^T] per column block; host divides rows 1:65 by row 0
    out_t = nc.dram_tensor("out_t", [H + 1, T], dt.float32, kind="ExternalOutput").ap()

    EXP = mybir.ActivationFunctionType.Exp
    DR = mybir.MatmulPerfMode.DoubleRow

    with tile.TileContext(nc) as tc:
        with (
            tc.tile_pool(name="consts", bufs=1) as consts,
            tc.tile_pool(name="inbuf", bufs=1) as inbuf,
            tc.tile_pool(name="proj", bufs=1) as proj,
            tc.tile_pool(name="projpsum", bufs=1, space="PSUM") as projpsum,
            tc.tile_pool(name="spsum", bufs=2, space="PSUM") as spsum,
            tc.tile_pool(name="opsum", bufs=1, space="PSUM") as opsum,
            tc.tile_pool(name="pbuf", bufs=3) as pbuf,
        ):
            # ---- constants: identity + weights FIRST on the sync ring -------
            idb_s = consts.tile([P, P], dt.bfloat16)
            mask_s = consts.tile([P, P], dt.bfloat16)
            wq_s = consts.tile([P, CCH, P], dt.float8e4)
            wkv_s = consts.tile([P, CCH, P], dt.bfloat16)
            nc.sync.dma_start(out=idb_s[:], in_=idb[:])
            nc.sync.dma_start(out=wkv_s[:], in_=wkv[:])

            kT_s = inbuf.tile([P, NB, CCH, 512], dt.bfloat16)
            qT_s = inbuf.tile([P, NB, CCH, 512], dt.float8e4)
            KVT_s = proj.tile([P, T], dt.bfloat16)   # rows 0:64 K^T, 64:128 V^T
            QT_s = proj.tile([P, T], dt.bfloat16)    # Q^T duplicated both halves
            KTD_s = proj.tile([P, T], dt.bfloat16)   # rows 64:128 = K^T dup
            V1_s = proj.tile([P, NJ, 66], dt.bfloat16)  # ones col + V natural
            O3_s = proj.tile([H + 1, 512], dt.float32)  # i-block 3 accumulator
            nc.vector.memset(V1_s[:, :, 0:1], 1.0)

            # ---- input DMAs: k0, wq, q0, q3 early; k3 last ------------------
            def dma_k(tb):
                nc.sync.dma_start(out=kT_s[:, tb, 0:4], in_=kT[tb, :, 0:4])
                nc.sync.dma_start(out=kT_s[:, tb, 4:8], in_=kT[tb, :, 4:8])

            def dma_q(tb):
                nc.sync.dma_start(out=qT_s[:, tb, 0:4], in_=qT[tb, :, 0:4])
                nc.sync.dma_start(out=qT_s[:, tb, 4:8], in_=qT[tb, :, 4:8])

            dma_k(0)
            nc.sync.dma_start(out=wq_s[:], in_=wq[:])
            dma_q(0)
            dma_q(3)
            nc.sync.dma_start(out=mask_s[:], in_=dmask[:])
            dma_k(1)
            dma_q(1)
            dma_k(2)
            dma_q(2)
            dma_k(3)

            # ---- HAM warm-up: ~3us of identity matmuls while k0 streams -----
            for w in range(NWARM):
                wp = projpsum.tile([P, 512], dt.float32, tag="kv")
                nc.tensor.matmul(wp[:, 0:P], lhsT=idb_s[:], rhs=idb_s[:],
                                 start=True, stop=True)

            # ---- pipeline stages --------------------------------------------
            def proj_kv(tb):
                """Project one 512-col block of k into K^T/V^T (+dup, +V tiles)."""
                sl = slice(512 * tb, 512 * (tb + 1))
                KVp = projpsum.tile([P, 512], dt.float32, tag="kv")
                for c in range(CCH):
                    nc.tensor.matmul(KVp[:], lhsT=wkv_s[:, c, :],
                                     rhs=kT_s[:, tb, c, :],
                                     start=(c == 0), stop=(c == CCH - 1))
                nc.vector.tensor_copy(out=KVT_s[:, sl], in_=KVp[:])
                # replicate K^T onto partitions 64:128 for row-tiled scores
                nc.scalar.dma_start(out=KTD_s[64:128, sl], in_=KVT_s[0:64, sl])
                for jj in range(4):
                    j = 4 * tb + jj
                    vtp = projpsum.tile([P, P], dt.bfloat16, tag="vt")
                    nc.tensor.transpose(
                        vtp[:], KVT_s[:, P * j:P * (j + 1)], idb_s[:])
                    nc.vector.tensor_copy(out=V1_s[:, j, 1:65], in_=vtp[:, 64:128])

            def proj_q(tb):
                """Project one 512-col block of q into Q^T (DoubleRow fp8)."""
                sl = slice(512 * tb, 512 * (tb + 1))
                Qp = projpsum.tile([P, 512], dt.float32, tag="q")
                for c2 in range(CCH // 2):
                    nc.tensor.matmul(Qp[:], lhsT=wq_s[:, 2 * c2:2 * c2 + 2, :],
                                     rhs=qT_s[:, tb, 2 * c2:2 * c2 + 2, :],
                                     perf_mode=DR,
                                     start=(c2 == 0), stop=(c2 == CCH // 2 - 1))
                nc.vector.tensor_copy(out=QT_s[:, sl], in_=Qp[:])

            def attn_pairs(ic, pairs, part_first, part_last):
                """Row-tiled score pairs + exp + PV accumulation for i-block ic.

                pairs: list of (jA, jB) or (jA, None).  part_first/part_last
                bound the PSUM accumulation group for this call."""
                ilo = 512 * ic
                ihi = 512 * (ic + 1)
                OUTp = opsum.tile([H + 1, 512], dt.float32, tag="out")
                for pi, (jA, jB) in enumerate(pairs):
                    loA = max(P * jA, ilo)
                    wA = ihi - loA
                    Sp = spsum.tile([P, 1024], dt.float32, tag="s")
                    nc.tensor.matmul(Sp[:, 0:wA],
                                     lhsT=KVT_s[0:H, P * jA:P * (jA + 1)],
                                     rhs=QT_s[0:H, loA:loA + wA],
                                     start=True, stop=True)
                    if jB is not None:
                        loB = max(P * jB, ilo)
                        wB = ihi - loB
                        nc.tensor.matmul(Sp[:, 512:512 + wB],
                                         lhsT=KTD_s[64:128, P * jB:P * (jB + 1)],
                                         rhs=QT_s[64:128, loB:loB + wB],
                                         start=True, stop=True)
                    else:
                        wB = 0
                    Pt = pbuf.tile([P, 1024], dt.bfloat16, tag="p")
                    nc.scalar.activation(out=Pt[:, 0:512 + wB],
                                         in_=Sp[:, 0:512 + wB],
                                         func=EXP, scale=SCALE / QS)
                    if jA >= 4 * ic:
                        nc.vector.tensor_mul(Pt[:, 0:P], Pt[:, 0:P], mask_s[:])
                    if jB is not None and jB >= 4 * ic:
                        nc.vector.tensor_mul(Pt[:, 512:512 + P],
                                             Pt[:, 512:512 + P], mask_s[:])
                    nc.tensor.matmul(OUTp[:, loA - ilo:512],
                                     lhsT=V1_s[:, jA, 0:65],
                                     rhs=Pt[:, 0:wA],
                                     start=(part_first and pi == 0), stop=False)
                    if jB is not None:
                        nc.tensor.matmul(OUTp[:, loB - ilo:512],
                                         lhsT=V1_s[:, jB, 0:65],
                                         rhs=Pt[:, 512:512 + wB],
                                         start=False,
                                         stop=(part_last and pi == len(pairs) - 1))
                    elif part_last and pi == len(pairs) - 1:
                        # close the group with a 0-width... not possible; the
                        # unpaired variant closes on its own last PV above.
                        pass
                return OUTp

            def attn_block(ic):
                """Full attention for i-block ic; unnormalized store via SBUF."""
                nj = 4 * ic + 4
                pairs = [(2 * p_, 2 * p_ + 1) for p_ in range(nj // 2)]
                OUTp = attn_pairs(ic, pairs, True, True)
                ob = pbuf.tile([H + 1, 512], dt.float32, tag="ob")
                nc.vector.tensor_copy(out=ob[:], in_=OUTp[:])
                nc.sync.dma_start(out=out_t[:, 512 * ic:512 * (ic + 1)],
                                  in_=ob[:])

            def attn3_part(pairs, first):
                """Spread part of i-block 3: accumulate into O3_s via DVE."""
                OUTp = attn_pairs(3, pairs, True, True)
                if first:
                    nc.vector.tensor_copy(out=O3_s[:], in_=OUTp[:])
                else:
                    nc.vector.tensor_add(O3_s[:], O3_s[:], OUTp[:])

            proj_kv(0)
            proj_q(0)
            attn_block(0)
            proj_q(3)
            attn3_part([(0, 1), (2, 3)], first=True)
            proj_kv(1)
            proj_q(1)
            attn_block(1)
            attn3_part([(4, 5), (6, 7)], first=False)
            proj_kv(2)
            proj_q(2)
            attn_block(2)
            attn3_part([(8, 9), (10, 11)], first=False)
            proj_kv(3)
            attn3_part([(12, 13), (14, 15)], first=False)
            nc.sync.dma_start(out=out_t[:, 1536:2048], in_=O3_s[:])

    nc.compile()
    return nc


def _get_nc():
    if "nc" not in _cached:
        _cached["nc"] = _build()
    return _cached["nc"]


def _block(xT, dtype):
    """[C, T] -> [NB, P, CCH, 512] so each 512-col block is contiguous."""
    return np.ascontiguousarray(
        xT.reshape(CCH, P, NB, 512).transpose(2, 1, 0, 3)).astype(dtype)


def _wblock(w, dtype):
    """[C, Hw] -> [P, CCH, Hw] contiguous (contraction chunks on partitions)."""
    return np.ascontiguousarray(
        w.reshape(CCH, P, w.shape[1]).transpose(1, 0, 2)).astype(dtype)


def _host_inputs(q, k, Wq, Wk, Wv):
    bf16 = ml_dtypes.bfloat16
    fp8 = ml_dtypes.float8_e4m3
    wq_h = _wblock(np.concatenate([Wq, Wq], axis=1) * QS, fp8)
    wkv_h = _wblock(np.concatenate([Wk, Wv], axis=1), bf16)
    dmask_h = np.triu(np.ones((P, P), dtype=np.float32)).astype(bf16)
    idb_h = np.eye(P, dtype=np.float32).astype(bf16)
    in_maps = []
    for b in range(B):
        in_maps.append({
            "qT": _block(q[b].T, fp8),
            "kT": _block(k[b].T, bf16),
            "wq": wq_h,
            "wkv": wkv_h,
            "dmask": dmask_h,
            "idb": idb_h,
        })
    return in_maps


def _unshard(res_b):
    o = res_b["out_t"]                      # [H+1, T] f32: row 0 = l
    return (o[1:H + 1] / o[0:1]).T          # [T, H]


def kernel(q, k, Wq, Wk, Wv):
    from concourse.bass_utils import run_bass_kernel_spmd

    nc = _get_nc()
    in_maps = _host_inputs(q, k, Wq, Wk, Wv)
    res = run_bass_kernel_spmd(nc, in_maps, list(range(B)))
    return np.stack([_unshard(res.results[b]) for b in range(B)]).astype(np.float32)


if __name__ == "__main__":
    rng = np.random.default_rng(0)
    q = rng.standard_normal((B, T, C)).astype(np.float32)
    k = rng.standard_normal((B, T, C)).astype(np.float32)
    Wq = (rng.standard_normal((C, H)) * 0.02).astype(np.float32)
    Wk = (rng.standard_normal((C, H)) * 0.02).astype(np.float32)
    Wv = (rng.standard_normal((C, H)) * 0.02).astype(np.float32)
    o = kernel(q, k, Wq, Wk, Wv)
    print("out", o.shape, o.dtype, float(np.abs(o).max()))


# revision 8
# speedup vs baseline: 1.0902x; 1.0209x over previous
"""Single-head causal attention (B=8, T=2048, C=1024, H=64) on 8 TRN2 NeuronCores.

Strategy: pure data parallelism — batch element b runs on core b. Each core
computes, for its [T, C] slices q_b / k_b:

    Q = q_b @ Wq ; K = k_b @ Wk ; V = k_b @ Wv          (projections)
    S = Q @ K^T / sqrt(C), causal-masked ; P = exp(S)    (no max-subtract:
    out = (P @ V) / (P @ 1)                               S is well-scaled)

Device-side layout:
  * Host pre-transposes q/k to [C, T] and pre-blocks them [tb, p, c, t];
    contraction (C) lands on SBUF partitions, zero on-chip input transposes.
    k ships bf16 (feeds K and V);  q ships fp8-e4m3 and Wq ships fp8
    pre-scaled by 64 (1/64 folded into the exp scale), so the Q projection
    runs in DoubleRow mode (2 contraction chunks per matmul, 2x rate).
  * Weights+identity are DMA'd FIRST; ~3us of identity warm-up matmuls run
    while the first k block streams in, so the PE HAM clock-gate reaches
    8/8 before real work starts.  DMA order retires the k3-dependent tail
    last (q3 early), and attention for the last i-block is spread through
    the kernel into an SBUF accumulator so the endgame exp burst is gone.
  * Q projection uses a duplicated stationary [Wq | Wq], so Q^T comes out
    replicated on partition halves 0:64 / 64:128 in one pass.  K^T is
    likewise replicated to partitions 64:128 by a small SBUF->SBUF DMA.
  * Scores run as ROW-TILED PAIRS: chunk for key-tile j uses PE rows 0:63
    (operands on partitions 0:64), chunk j+1 rows 64:127 (operands on
    partitions 64:128); the two matmuls execute concurrently (contraction
    is only H=64), doubling score throughput.  One wide scalar-engine
    activation exps both PSUM banks of a pair.
  * Softmax denominators come free via a ones column appended to V (row 0
    of the PV accumulator is P @ 1).  Outputs leave PSUM by direct
    PSUM->DRAM DMA, unnormalized [l; out^T] fp32; the host divides.
"""

import numpy as np
import ml_dtypes

B, T, C, H = 8, 2048, 1024, 64
P = 128                  # SBUF partitions
CCH = C // P             # 8 contraction chunks
NJ = T // P              # 16 key tiles of 128
NB = T // 512            # 4 column blocks of 512
SCALE = float(C) ** -0.5
QS = 64.0                # fp8 Wq pre-scale (folded out via exp scale)
NWARM = 8                # HAM warm-up matmuls (wide, on wkv)

_cached = {}


def _build():
    import concourse.bass as bass
    import concourse.mybir as mybir
    import concourse.tile as tile
    from concourse import bacc

    dt = mybir.dt
    nc = bacc.Bacc("TRN2", target_bir_lowering=False, debug=False, num_devices=B)

    qT = nc.dram_tensor("qT", [NB, P, CCH, 512], dt.float8e4, kind="ExternalInput").ap()
    kT = nc.dram_tensor("kT", [NB, P, CCH, 512], dt.bfloat16, kind="ExternalInput").ap()
    wq = nc.dram_tensor("wq", [P, CCH, P], dt.float8e4, kind="ExternalInput").ap()
    wkv = nc.dram_tensor("wkv", [P, CCH, P], dt.bfloat16, kind="ExternalInput").ap()
    dmask = nc.dram_tensor("dmask", [P, P], dt.bfloat16, kind="ExternalInput").ap()
    idb = nc.dram_tensor("idb", [P, P], dt.bfloat16, kind="ExternalInput").ap()
    # unnormalized [l ; out^T] per column block; host divides rows 1:65 by row 0
    out_t = nc.dram_tensor("out_t", [H + 1, T], dt.float32, kind="ExternalOutput").ap()

    EXP = mybir.ActivationFunctionType.Exp
    DR = mybir.MatmulPerfMode.DoubleRow

    with tile.TileContext(nc) as tc:
        with (
            tc.tile_pool(name="consts", bufs=1) as consts,
            tc.tile_pool(name="inbuf", bufs=1) as inbuf,
            tc.tile_pool(name="proj", bufs=1) as proj,
            tc.tile_pool(name="projpsum", bufs=1, space="PSUM") as projpsum,
            tc.tile_pool(name="spsum", bufs=2, space="PSUM") as spsum,
            tc.tile_pool(name="opsum", bufs=1, space="PSUM") as opsum,
            tc.tile_pool(name="pbuf", bufs=3) as pbuf,
        ):
            # ---- constants: identity + weights FIRST on the sync ring -------
            idb_s = consts.tile([P, P], dt.bfloat16)
            mask_s = consts.tile([P, P], dt.bfloat16)
            wq_s = consts.tile([P, CCH, P], dt.float8e4)
            wkv_s = consts.tile([P, CCH, P], dt.bfloat16)
            nc.sync.dma_start(out=idb_s[:], in_=idb[:])
            nc.sync.dma_start(out=wkv_s[:], in_=wkv[:])

            kT_s = inbuf.tile([P, NB, CCH, 512], dt.bfloat16)
            qT_s = inbuf.tile([P, NB, CCH, 512], dt.float8e4)
            KVT_s = proj.tile([P, T], dt.bfloat16)   # rows 0:64 K^T, 64:128 V^T
            QT_s = proj.tile([P, T], dt.bfloat16)    # Q^T duplicated both halves
            KTD_s = proj.tile([P, T], dt.bfloat16)   # rows 64:128 = K^T dup
            V1_s = proj.tile([P, NJ, 66], dt.bfloat16)  # ones col + V natural
            O3_s = proj.tile([H + 1, 512], dt.float32)  # i-block 3 accumulator
            nc.vector.memset(V1_s[:, :, 0:1], 1.0)

            # ---- input DMAs: k0, wq, q0, k1 early; q3 next; k3 last ---------
            def dma_k(tb):
                nc.sync.dma_start(out=kT_s[:, tb, 0:4], in_=kT[tb, :, 0:4])
                nc.sync.dma_start(out=kT_s[:, tb, 4:8], in_=kT[tb, :, 4:8])

            def dma_q(tb):
                nc.sync.dma_start(out=qT_s[:, tb], in_=qT[tb, :])

            dma_k(0)
            nc.sync.dma_start(out=wq_s[:], in_=wq[:])
            dma_q(0)
            nc.sync.dma_start(out=mask_s[:], in_=dmask[:])
            dma_k(1)
            dma_q(3)
            dma_q(1)
            dma_k(2)
            dma_q(2)
            dma_k(3)

            # ---- HAM warm-up: dense matmuls while k0 streams ----------------
            # (a few short identity matmuls as soon as idb lands, then wide
            # ones on the wkv tile — the goal is sustained PE activity so the
            # clock gate reaches 8/8 before the first real projection)
            for w in range(4):
                wp = projpsum.tile([P, 512], dt.float32, tag="kv")
                nc.tensor.matmul(wp[:, 0:P], lhsT=idb_s[:], rhs=idb_s[:],
                                 start=True, stop=True)
            for w in range(NWARM):
                wp = projpsum.tile([P, 512], dt.float32, tag="kv")
                nc.tensor.matmul(wp[:], lhsT=idb_s[:],
                                 rhs=wkv_s[:, 4 * (w % 2):4 * (w % 2) + 4, :],
                                 start=True, stop=True)

            # ---- pipeline stages --------------------------------------------
            def proj_kv(tb):
                """Project one 512-col block of k into K^T/V^T (+dup, +V tiles)."""
                sl = slice(512 * tb, 512 * (tb + 1))
                KVp = projpsum.tile([P, 512], dt.float32, tag="kv")
                for c in range(CCH):
                    nc.tensor.matmul(KVp[:], lhsT=wkv_s[:, c, :],
                                     rhs=kT_s[:, tb, c, :],
                                     start=(c == 0), stop=(c == CCH - 1))
                nc.vector.tensor_copy(out=KVT_s[:, sl], in_=KVp[:])
                # replicate K^T onto partitions 64:128 for row-tiled scores
                nc.scalar.dma_start(out=KTD_s[64:128, sl], in_=KVT_s[0:64, sl])
                for jj in range(4):
                    j = 4 * tb + jj
                    vtp = projpsum.tile([P, P], dt.bfloat16, tag="vt")
                    nc.tensor.transpose(
                        vtp[:], KVT_s[:, P * j:P * (j + 1)], idb_s[:])
                    nc.vector.tensor_copy(out=V1_s[:, j, 1:65], in_=vtp[:, 64:128])

            def proj_q(tb):
                """Project one 512-col block of q into Q^T (DoubleRow fp8)."""
                sl = slice(512 * tb, 512 * (tb + 1))
                Qp = projpsum.tile([P, 512], dt.float32, tag="q")
                for c2 in range(CCH // 2):
                    nc.tensor.matmul(Qp[:], lhsT=wq_s[:, 2 * c2:2 * c2 + 2, :],
                                     rhs=qT_s[:, tb, 2 * c2:2 * c2 + 2, :],
                                     perf_mode=DR,
                                     start=(c2 == 0), stop=(c2 == CCH // 2 - 1))
                nc.vector.tensor_copy(out=QT_s[:, sl], in_=Qp[:])

            def attn_pairs(ic, pairs, part_first, part_last):
                """Row-tiled score pairs + exp + PV accumulation for i-block ic.

                pairs: list of (jA, jB) or (jA, None).  part_first/part_last
                bound the PSUM accumulation group for this call."""
                ilo = 512 * ic
                ihi = 512 * (ic + 1)
                OUTp = opsum.tile([H + 1, 512], dt.float32, tag="out")
                for pi, (jA, jB) in enumerate(pairs):
                    loA = max(P * jA, ilo)
                    wA = ihi - loA
                    Sp = spsum.tile([P, 1024], dt.float32, tag="s")
                    nc.tensor.matmul(Sp[:, 0:wA],
                                     lhsT=KVT_s[0:H, P * jA:P * (jA + 1)],
                                     rhs=QT_s[0:H, loA:loA + wA],
                                     start=True, stop=True)
                    if jB is not None:
                        loB = max(P * jB, ilo)
                        wB = ihi - loB
                        nc.tensor.matmul(Sp[:, 512:512 + wB],
                                         lhsT=KTD_s[64:128, P * jB:P * (jB + 1)],
                                         rhs=QT_s[64:128, loB:loB + wB],
                                         start=True, stop=True)
                    else:
                        wB = 0
                    Pt = pbuf.tile([P, 1024], dt.bfloat16, tag="p")
                    nc.scalar.activation(out=Pt[:, 0:512 + wB],
                                         in_=Sp[:, 0:512 + wB],
                                         func=EXP, scale=SCALE / QS)
                    if jA >= 4 * ic:
                        nc.vector.tensor_mul(Pt[:, 0:P], Pt[:, 0:P], mask_s[:])
                    if jB is not None and jB >= 4 * ic:
                        nc.vector.tensor_mul(Pt[:, 512:512 + P],
                                             Pt[:, 512:512 + P], mask_s[:])
                    nc.tensor.matmul(OUTp[:, loA - ilo:512],
                                     lhsT=V1_s[:, jA, 0:65],
                                     rhs=Pt[:, 0:wA],
                                     start=(part_first and pi == 0), stop=False)
                    if jB is not None:
                        nc.tensor.matmul(OUTp[:, loB - ilo:512],
                                         lhsT=V1_s[:, jB, 0:65],
                                         rhs=Pt[:, 512:512 + wB],
                                         start=False,
                                         stop=(part_last and pi == len(pairs) - 1))
                    elif part_last and pi == len(pairs) - 1:
                        # close the group with a 0-width... not possible; the
                        # unpaired variant closes on its own last PV above.
                        pass
                return OUTp

            def attn_block(ic):
                """Full attention for i-block ic; unnormalized store via SBUF."""
                nj = 4 * ic + 4
                pairs = [(2 * p_, 2 * p_ + 1) for p_ in range(nj // 2)]
                OUTp = attn_pairs(ic, pairs, True, True)
                ob = pbuf.tile([H + 1, 512], dt.float32, tag="ob")
                nc.vector.tensor_copy(out=ob[:], in_=OUTp[:])
                nc.sync.dma_start(out=out_t[:, 512 * ic:512 * (ic + 1)],
                                  in_=ob[:])

            def attn3_part(pairs, first):
                """Spread part of i-block 3: accumulate into O3_s via DVE."""
                OUTp = attn_pairs(3, pairs, True, True)
                if first:
                    nc.vector.tensor_copy(out=O3_s[:], in_=OUTp[:])
                else:
                    nc.vector.tensor_add(O3_s[:], O3_s[:], OUTp[:])

            proj_kv(0)
            proj_q(0)
            attn_block(0)
            proj_kv(1)
            proj_q(3)
            attn3_part([(0, 1), (2, 3)], first=True)
            proj_q(1)
            attn_block(1)
            attn3_part([(4, 5), (6, 7)], first=False)
            proj_kv(2)
            proj_q(2)
            attn_block(2)
            attn3_part([(8, 9), (10, 11)], first=False)
            proj_kv(3)
            attn3_part([(12, 13), (14, 15)], first=False)
            nc.sync.dma_start(out=out_t[:, 1536:2048], in_=O3_s[:])

    nc.compile()
    return nc


def _get_nc():
    if "nc" not in _cached:
        _cached["nc"] = _build()
    return _cached["nc"]


def _block(xT, dtype):
    """[C, T] -> [NB, P, CCH, 512] so each 512-col block is contiguous."""
    return np.ascontiguousarray(
        xT.reshape(CCH, P, NB, 512).transpose(2, 1, 0, 3)).astype(dtype)


def _wblock(w, dtype):
    """[C, Hw] -> [P, CCH, Hw] contiguous (contraction chunks on partitions)."""
    return np.ascontiguousarray(
        w.reshape(CCH, P, w.shape[1]).transpose(1, 0, 2)).astype(dtype)


def _host_inputs(q, k, Wq, Wk, Wv):
    bf16 = ml_dtypes.bfloat16
    fp8 = ml_dtypes.float8_e4m3
    wq_h = _wblock(np.concatenate([Wq, Wq], axis=1) * QS, fp8)
    wkv_h = _wblock(np.concatenate([Wk, Wv], axis=1), bf16)
    dmask_h = np.triu(np.ones((P, P), dtype=np.float32)).astype(bf16)
    idb_h = np.eye(P, dtype=np.float32).astype(bf16)
    in_maps = []
    for b in range(B):
        in_maps.append({
            "qT": _block(q[b].T, fp8),
            "kT": _block(k[b].T, bf16),
            "wq": wq_h,
            "wkv": wkv_h,
            "dmask": dmask_h,
            "idb": idb_h,
        })
    return in_maps


def _unshard(res_b):
    o = res_b["out_t"]                      # [H+1, T] f32: row 0 = l
    return (o[1:H + 1] / o[0:1]).T          # [T, H]


def kernel(q, k, Wq, Wk, Wv):
    from concourse.bass_utils import run_bass_kernel_spmd

    nc = _get_nc()
    in_maps = _host_inputs(q, k, Wq, Wk, Wv)
    res = run_bass_kernel_spmd(nc, in_maps, list(range(B)))
    return np.stack([_unshard(res.results[b]) for b in range(B)]).astype(np.float32)


if __name__ == "__main__":
    rng = np.random.default_rng(0)
    q = rng.standard_normal((B, T, C)).astype(np.float32)
    k = rng.standard_normal((B, T, C)).astype(np.float32)
    Wq = (rng.standard_normal((C, H)) * 0.02).astype(np.float32)
    Wk = (rng.standard_normal((C, H)) * 0.02).astype(np.float32)
    Wv = (rng.standard_normal((C, H)) * 0.02).astype(np.float32)
    o = kernel(q, k, Wq, Wk, Wv)
    print("out", o.shape, o.dtype, float(np.abs(o).max()))


# revision 14
# speedup vs baseline: 1.1053x; 1.0138x over previous
"""Single-head causal attention (B=8, T=2048, C=1024, H=64) on 8 TRN2 NeuronCores.

Strategy: pure data parallelism — batch element b runs on core b. Each core
computes, for its [T, C] slices q_b / k_b:

    Q = q_b @ Wq ; K = k_b @ Wk ; V = k_b @ Wv          (projections)
    S = Q @ K^T / sqrt(C), causal-masked ; P = exp(S)    (no max-subtract:
    out = (P @ V) / (P @ 1)                               S is well-scaled)

Device-side layout:
  * Host pre-transposes q/k to [C, T] and pre-blocks them [tb, p, c, t];
    contraction (C) lands on SBUF partitions, zero on-chip input transposes.
    k ships bf16 (feeds K and V);  q ships fp8-e4m3 and Wq ships fp8
    pre-scaled by 64 (1/64 folded into the exp scale), so the Q projection
    runs in DoubleRow mode (2 contraction chunks per matmul, 2x rate).
  * Weights+identity are DMA'd FIRST; ~3us of identity warm-up matmuls run
    while the first k block streams in, so the PE HAM clock-gate reaches
    8/8 before real work starts.  DMA order retires the k3-dependent tail
    last (q3 early), and attention for the last i-block is spread through
    the kernel into an SBUF accumulator so the endgame exp burst is gone.
  * Q projection uses a duplicated stationary [Wq | Wq], so Q^T comes out
    replicated on partition halves 0:64 / 64:128 in one pass.  K^T is
    likewise replicated to partitions 64:128 by a small SBUF->SBUF DMA.
  * Scores run as ROW-TILED PAIRS: chunk for key-tile j uses PE rows 0:63
    (operands on partitions 0:64), chunk j+1 rows 64:127 (operands on
    partitions 64:128); the two matmuls execute concurrently (contraction
    is only H=64), doubling score throughput.  One wide scalar-engine
    activation exps both PSUM banks of a pair.
  * Softmax denominators come free via a ones column appended to V (row 0
    of the PV accumulator is P @ 1).  Outputs leave PSUM by direct
    PSUM->DRAM DMA, unnormalized [l; out^T] fp32; the host divides.
"""

import numpy as np
import ml_dtypes

B, T, C, H = 8, 2048, 1024, 64
P = 128                  # SBUF partitions
CCH = C // P             # 8 contraction chunks
NJ = T // P              # 16 key tiles of 128
NB = T // 512            # 4 column blocks of 512
SCALE = float(C) ** -0.5
QS = 64.0                # fp8 Wq pre-scale (folded out via exp scale)
NWARM = 8                # HAM warm-up matmuls (wide, on wkv)

_cached = {}


def _build():
    import concourse.bass as bass
    import concourse.mybir as mybir
    import concourse.tile as tile
    from concourse import bacc

    dt = mybir.dt
    nc = bacc.Bacc("TRN2", target_bir_lowering=False, debug=False, num_devices=B)

    qT = nc.dram_tensor("qT", [NB, P, CCH, 512], dt.float8e4, kind="ExternalInput").ap()
    kT = nc.dram_tensor("kT", [NB, P, CCH, 512], dt.bfloat16, kind="ExternalInput").ap()
    wq = nc.dram_tensor("wq", [P, CCH, P], dt.float8e4, kind="ExternalInput").ap()
    wkv = nc.dram_tensor("wkv", [P, CCH, P], dt.bfloat16, kind="ExternalInput").ap()
    dmask = nc.dram_tensor("dmask", [P, P], dt.bfloat16, kind="ExternalInput").ap()
    idb = nc.dram_tensor("idb", [P, P], dt.bfloat16, kind="ExternalInput").ap()
    shf = nc.dram_tensor("shf", [P, P], dt.bfloat16, kind="ExternalInput").ap()
    # unnormalized [l ; out^T] per column block; host divides rows 1:65 by row 0
    out_t = nc.dram_tensor("out_t", [H + 1, T], dt.float32, kind="ExternalOutput").ap()

    EXP = mybir.ActivationFunctionType.Exp
    DR = mybir.MatmulPerfMode.DoubleRow

    with tile.TileContext(nc) as tc:
        with (
            tc.tile_pool(name="consts", bufs=1) as consts,
            tc.tile_pool(name="inbuf", bufs=1) as inbuf,
            tc.tile_pool(name="proj", bufs=1) as proj,
            tc.tile_pool(name="projpsum", bufs=1, space="PSUM") as projpsum,
            tc.tile_pool(name="spsum", bufs=2, space="PSUM") as spsum,
            tc.tile_pool(name="opsum", bufs=1, space="PSUM") as opsum,
            tc.tile_pool(name="pbuf", bufs=3) as pbuf,
        ):
            # ---- constants: identity + weights FIRST on the sync ring -------
            idb_s = consts.tile([P, P], dt.bfloat16)
            shf_s = consts.tile([P, P], dt.bfloat16)
            mask_s = consts.tile([P, P], dt.bfloat16)
            wq_s = consts.tile([P, CCH, P], dt.float8e4)
            wkv_s = consts.tile([P, CCH, P], dt.bfloat16)
            nc.sync.dma_start(out=idb_s[:], in_=idb[:])
            nc.sync.dma_start(out=shf_s[:], in_=shf[:])
            nc.sync.dma_start(out=wkv_s[:], in_=wkv[:])

            kT_s = inbuf.tile([P, NB, CCH, 512], dt.bfloat16)
            qT_s = inbuf.tile([P, NB, CCH, 512], dt.float8e4)
            KVT_s = proj.tile([P, T], dt.bfloat16)   # rows 0:64 K^T, 64:128 V^T
            QT_s = proj.tile([P, T], dt.bfloat16)    # Q^T duplicated both halves
            KTD_s = proj.tile([P, T], dt.bfloat16)   # rows 64:128 = K^T dup
            V1_s = proj.tile([P, NJ, 66], dt.bfloat16)  # ones col + V natural
            O3_s = proj.tile([H + 1, 512], dt.float32)  # i-block 3 accumulator
            nc.vector.memset(V1_s[:, :, 0:1], 1.0)

            # ---- input DMAs: k0, wq, q0, k1 early; q3 next; k3 last ---------
            def dma_k(tb):
                nc.sync.dma_start(out=kT_s[:, tb, 0:4], in_=kT[tb, :, 0:4])
                nc.sync.dma_start(out=kT_s[:, tb, 4:8], in_=kT[tb, :, 4:8])

            def dma_q(tb):
                nc.sync.dma_start(out=qT_s[:, tb], in_=qT[tb, :])

            dma_k(0)
            nc.sync.dma_start(out=wq_s[:], in_=wq[:])
            dma_q(0)
            nc.sync.dma_start(out=mask_s[:], in_=dmask[:])
            dma_k(1)
            dma_q(3)
            dma_q(1)
            dma_k(2)
            dma_q(2)
            dma_k(3)

            # ---- HAM warm-up: dense matmuls while k0 streams ----------------
            # (a few short identity matmuls as soon as idb lands, then wide
            # ones on the wkv tile — the goal is sustained PE activity so the
            # clock gate reaches 8/8 before the first real projection)
            for w in range(4):
                wp = projpsum.tile([P, 512], dt.float32, tag="kv" if w % 2 else "q")
                nc.tensor.matmul(wp[:, 0:P], lhsT=idb_s[:], rhs=idb_s[:],
                                 start=True, stop=True)
            for w in range(NWARM):
                wp = projpsum.tile([P, 512], dt.float32, tag="kv" if w % 2 else "q")
                nc.tensor.matmul(wp[:], lhsT=idb_s[:],
                                 rhs=wkv_s[:, 4 * (w % 2):4 * (w % 2) + 4, :],
                                 start=True, stop=True)

            # ---- pipeline stages --------------------------------------------
            def proj_kv(tb):
                """Project one 512-col block of k into K^T/V^T (+dup, +V tiles)."""
                sl = slice(512 * tb, 512 * (tb + 1))
                KVp = projpsum.tile([P, 512], dt.float32, tag="kv")
                for c in range(CCH):
                    nc.tensor.matmul(KVp[:], lhsT=wkv_s[:, c, :],
                                     rhs=kT_s[:, tb, c, :],
                                     start=(c == 0), stop=(c == CCH - 1))
                nc.vector.tensor_copy(out=KVT_s[:, sl], in_=KVp[:])
                # replicate K^T onto partitions 64:128 for row-tiled scores:
                # PE shift-matmul (out[64+i,:] = K^T[i,:]) + small DVE copy —
                # a DMA here would crawl behind the streaming input transfers
                KDp = projpsum.tile([P, 512], dt.float32, tag="q")
                nc.tensor.matmul(KDp[:], lhsT=shf_s[0:64, :],
                                 rhs=KVT_s[0:64, sl], start=True, stop=True)
                nc.vector.tensor_copy(out=KTD_s[64:128, sl], in_=KDp[64:128, :])
                for jj in range(4):
                    j = 4 * tb + jj
                    vtp = projpsum.tile([P, P], dt.bfloat16, tag="vt")
                    nc.tensor.transpose(
                        vtp[:], KVT_s[:, P * j:P * (j + 1)], idb_s[:])
                    nc.vector.tensor_copy(out=V1_s[:, j, 1:65], in_=vtp[:, 64:128])

            def proj_q(tb):
                """Project one 512-col block of q into Q^T (DoubleRow fp8)."""
                sl = slice(512 * tb, 512 * (tb + 1))
                Qp = projpsum.tile([P, 512], dt.float32, tag="q")
                for c2 in range(CCH // 2):
                    nc.tensor.matmul(Qp[:], lhsT=wq_s[:, 2 * c2:2 * c2 + 2, :],
                                     rhs=qT_s[:, tb, 2 * c2:2 * c2 + 2, :],
                                     perf_mode=DR,
                                     start=(c2 == 0), stop=(c2 == CCH // 2 - 1))
                nc.vector.tensor_copy(out=QT_s[:, sl], in_=Qp[:])

            def attn_pairs(ic, pairs, part_first, part_last):
                """Row-tiled score pairs + exp + PV accumulation for i-block ic.

                pairs: list of (jA, jB) or (jA, None).  part_first/part_last
                bound the PSUM accumulation group for this call."""
                ilo = 512 * ic
                ihi = 512 * (ic + 1)
                OUTp = opsum.tile([H + 1, 512], dt.float32, tag="out")
                for pi, (jA, jB) in enumerate(pairs):
                    loA = max(P * jA, ilo)
                    wA = ihi - loA
                    Sp = spsum.tile([P, 1024], dt.float32, tag="s")
                    nc.tensor.matmul(Sp[:, 0:wA],
                                     lhsT=KVT_s[0:H, P * jA:P * (jA + 1)],
                                     rhs=QT_s[0:H, loA:loA + wA],
                                     start=True, stop=True)
                    if jB is not None:
                        loB = max(P * jB, ilo)
                        wB = ihi - loB
                        nc.tensor.matmul(Sp[:, 512:512 + wB],
                                         lhsT=KTD_s[64:128, P * jB:P * (jB + 1)],
                                         rhs=QT_s[64:128, loB:loB + wB],
                                         start=True, stop=True)
                    else:
                        wB = 0
                    Pt = pbuf.tile([P, 1024], dt.bfloat16, tag="p")
                    nc.scalar.activation(out=Pt[:, 0:512 + wB],
                                         in_=Sp[:, 0:512 + wB],
                                         func=EXP, scale=SCALE / QS)
                    if jA >= 4 * ic:
                        nc.vector.tensor_mul(Pt[:, 0:P], Pt[:, 0:P], mask_s[:])
                    if jB is not None and jB >= 4 * ic:
                        nc.vector.tensor_mul(Pt[:, 512:512 + P],
                                             Pt[:, 512:512 + P], mask_s[:])
                    nc.tensor.matmul(OUTp[:, loA - ilo:512],
                                     lhsT=V1_s[:, jA, 0:65],
                                     rhs=Pt[:, 0:wA],
                                     start=(part_first and pi == 0), stop=False)
                    if jB is not None:
                        nc.tensor.matmul(OUTp[:, loB - ilo:512],
                                         lhsT=V1_s[:, jB, 0:65],
                                         rhs=Pt[:, 512:512 + wB],
                                         start=False,
                                         stop=(part_last and pi == len(pairs) - 1))
                    elif part_last and pi == len(pairs) - 1:
                        # close the group with a 0-width... not possible; the
                        # unpaired variant closes on its own last PV above.
                        pass
                return OUTp

            def attn_block(ic):
                """Full attention for i-block ic; unnormalized store via SBUF."""
                nj = 4 * ic + 4
                pairs = [(2 * p_, 2 * p_ + 1) for p_ in range(nj // 2)]
                OUTp = attn_pairs(ic, pairs, True, True)
                ob = pbuf.tile([H + 1, 512], dt.float32, tag="ob")
                nc.vector.tensor_copy(out=ob[:], in_=OUTp[:])
                nc.sync.dma_start(out=out_t[:, 512 * ic:512 * (ic + 1)],
                                  in_=ob[:])

            def attn3_part(pairs, first):
                """Spread part of i-block 3: accumulate into O3_s via DVE."""
                OUTp = attn_pairs(3, pairs, True, True)
                if first:
                    nc.vector.tensor_copy(out=O3_s[:], in_=OUTp[:])
                else:
                    nc.vector.tensor_add(O3_s[:], O3_s[:], OUTp[:])

            proj_kv(0)
            proj_q(0)
            attn_block(0)
            proj_kv(1)
            proj_q(3)
            attn3_part([(0, 1), (2, 3)], first=True)
            proj_q(1)
            attn_block(1)
            attn3_part([(4, 5), (6, 7)], first=False)
            proj_kv(2)
            proj_q(2)
            attn_block(2)
            attn3_part([(8, 9), (10, 11)], first=False)
            proj_kv(3)
            attn3_part([(12, 13), (14, 15)], first=False)
            nc.sync.dma_start(out=out_t[:, 1536:2048], in_=O3_s[:])

    nc.compile()
    return nc


def _get_nc():
    if "nc" not in _cached:
        _cached["nc"] = _build()
    return _cached["nc"]


def _block(xT, dtype):
    """[C, T] -> [NB, P, CCH, 512] so each 512-col block is contiguous."""
    return np.ascontiguousarray(
        xT.reshape(CCH, P, NB, 512).transpose(2, 1, 0, 3)).astype(dtype)


def _wblock(w, dtype):
    """[C, Hw] -> [P, CCH, Hw] contiguous (contraction chunks on partitions)."""
    return np.ascontiguousarray(
        w.reshape(CCH, P, w.shape[1]).transpose(1, 0, 2)).astype(dtype)


def _host_inputs(q, k, Wq, Wk, Wv):
    bf16 = ml_dtypes.bfloat16
    fp8 = ml_dtypes.float8_e4m3
    wq_h = _wblock(np.concatenate([Wq, Wq], axis=1) * QS, fp8)
    wkv_h = _wblock(np.concatenate([Wk, Wv], axis=1), bf16)
    dmask_h = np.triu(np.ones((P, P), dtype=np.float32)).astype(bf16)
    idb_h = np.eye(P, dtype=np.float32).astype(bf16)
    shf_h = np.zeros((P, P), dtype=np.float32)
    shf_h[np.arange(64), 64 + np.arange(64)] = 1.0   # out[64+i] = in[i]
    shf_h = shf_h.astype(bf16)
    in_maps = []
    for b in range(B):
        in_maps.append({
            "qT": _block(q[b].T, fp8),
            "kT": _block(k[b].T, bf16),
            "wq": wq_h,
            "wkv": wkv_h,
            "dmask": dmask_h,
            "idb": idb_h,
            "shf": shf_h,
        })
    return in_maps


def _unshard(res_b):
    o = res_b["out_t"]                      # [H+1, T] f32: row 0 = l
    return (o[1:H + 1] / o[0:1]).T          # [T, H]


def kernel(q, k, Wq, Wk, Wv):
    from concourse.bass_utils import run_bass_kernel_spmd

    nc = _get_nc()
    in_maps = _host_inputs(q, k, Wq, Wk, Wv)
    res = run_bass_kernel_spmd(nc, in_maps, list(range(B)))
    return np.stack([_unshard(res.results[b]) for b in range(B)]).astype(np.float32)


if __name__ == "__main__":
    rng = np.random.default_rng(0)
    q = rng.standard_normal((B, T, C)).astype(np.float32)
    k = rng.standard_normal((B, T, C)).astype(np.float32)
    Wq = (rng.standard_normal((C, H)) * 0.02).astype(np.float32)
    Wk = (rng.standard_normal((C, H)) * 0.02).astype(np.float32)
    Wv = (rng.standard_normal((C, H)) * 0.02).astype(np.float32)
    o = kernel(q, k, Wq, Wk, Wv)
    print("out", o.shape, o.dtype, float(np.abs(o).max()))
